# revision 10
# baseline (speedup 1.0000x reference)
"""Trainium2 Bass kernel: vLLM-style last-token KV-cache scatter.

Reference semantics (CacheOnlyAttentionLayer):
  last  = clip(query_start_loc[1:num_reqs+1] - 1, 0, T-1)
  kv    = hidden_states[last].reshape(R, 2, Hkv, D)
  slots = slot_mapping[last]; blk = slots // BS; off = slots % BS
  out   = kv_cache.at[0, blk, off].set(kv[:,0]).at[1, blk, off].set(kv[:,1])

The output is the full (2, 4096, 16, 8, 128) f32 cache (512 MiB): a copy of
kv_cache with <=512 scattered 4 KiB rows overwritten.  Memory-bound: the
intrinsic work is the cache copy.

Distribution: shard the cache by block index across 8 cores (each core owns
512 blocks = 64 MiB).  Host routes each (row, value) update to its owning
core; the device kernel bulk-copies its shard DRAM->DRAM and applies its
updates with indirect (scatter) DMAs.  One SPMD program for all cores;
per-core update tables arrive as input tensors, padded with idempotent
duplicate writes so no bounds checking or control flow is needed.

Update layout: groups 0..G/2-1 hold key-plane rows (< 8192), groups
G/2..G-1 hold value-plane rows (>= 8192).  The key scatter fires as soon as
the key-plane chunks have copied, overlapping the value-plane copy.
"""

import numpy as np

import concourse.bass as bass
import concourse.mybir as mybir
from concourse import bass_utils

# Problem constants (hardcoded per contract; kernel.py must be self-contained).
NUM_KV_HEADS = 8
HEAD_SIZE = 128
BLOCK_SIZE = 16
NUM_BLOCKS = 4096
TOTAL_TOKENS = 32768
HIDDEN = 2 * NUM_KV_HEADS * HEAD_SIZE  # 2048
ROW = NUM_KV_HEADS * HEAD_SIZE  # 1024 f32 = 4 KiB: one (plane, block, offset) row

N_CORES = 8
BLOCKS_PER_CORE = NUM_BLOCKS // N_CORES  # 512
PLANE_ROWS = BLOCKS_PER_CORE * BLOCK_SIZE  # 8192 rows per key/value plane
ROWS_PER_CORE = 2 * PLANE_ROWS  # 16384 rows of ROW f32 = 64 MiB
MAX_UPD = 512  # worst case: all 256 requests (key+value rows) on one core
UPD_GROUPS = MAX_UPD // 128  # indirect-DMA calls of 128 rows each
KEY_GROUPS = UPD_GROUPS // 2  # groups carrying key-plane rows

# Tuning knobs (bench.py overrides these before building the program).
# Each copy stream is a DMA ring ("sync"/"scalar" = the two HWDGE rings,
# "gpsimd" = SWDGE); every stream copies one contiguous segment of each
# plane, so each ring carries exactly 2 big DMAs.  Hardware-measured: >2
# queued DMAs per ring hits a ~35 us inter-DMA stall, so keep it at 2.
COPY_STREAMS = ("sync", "scalar")
SPLIT_SCATTER = True  # fire key-plane scatter after key-plane segments
NO_GPSIMD_DRAIN = True  # skip Q7 dge_drain in the block exit barrier
PRESCATTER_VALUE = True  # scatter value rows into cache_in before its copy

# Module-level caches so repeat kernel() calls reuse the compiled program.
_NC = None
_NC_KEY = None

# Set by the test harness to profile: {"trace": True, "trace_cores": [...]}.
RUN_KWARGS: dict = {}
LAST_RESULTS = None


def _build_program():
    """One SPMD Bass program; all 8 cores run it on their own shard."""
    nc = bass.Bass("TRN2", debug=False)

    cache_in = nc.dram_tensor(
        "cache_in", [ROWS_PER_CORE, ROW], mybir.dt.float32, kind="ExternalInput"
    )
    upd = nc.dram_tensor(
        "upd", [128, UPD_GROUPS * ROW], mybir.dt.float32, kind="ExternalInput"
    )
    idx = nc.dram_tensor(
        "idx", [128, UPD_GROUPS], mybir.dt.int32, kind="ExternalInput"
    )
    cache_out = nc.dram_tensor(
        "cache_out", [ROWS_PER_CORE, ROW], mybir.dt.float32, kind="ExternalOutput"
    )

    # Each stream copies one contiguous segment of each plane (as equal as
    # possible); segment boundaries land on row granularity.
    n_str = len(COPY_STREAMS)
    cuts = np.linspace(0, PLANE_ROWS, n_str + 1).astype(int)
    # per engine name -> list of (r0, r1) spans, key plane then value plane
    spans = {}
    for k, name in enumerate(COPY_STREAMS):
        spans.setdefault(name, []).append(("k", int(cuts[k]), int(cuts[k + 1])))
    for k, name in enumerate(COPY_STREAMS):
        spans.setdefault(name, []).append(
            ("v", PLANE_ROWS + int(cuts[k]), PLANE_ROWS + int(cuts[k + 1]))
        )

    with (
        nc.sbuf_tensor([128, UPD_GROUPS * ROW], mybir.dt.float32) as upd_sb,
        nc.sbuf_tensor([128, UPD_GROUPS], mybir.dt.int32) as idx_sb,
        nc.semaphore() as copyk_sem,  # completed key-plane segments (x16)
        nc.semaphore() as copyv_sem,  # completed value-plane segments (x16)
        nc.semaphore() as load_sem,
        nc.semaphore() as scatv_sem,  # value rows pre-scattered to cache_in
        nc.semaphore() as scat_sem,
        nc.Block(no_gpsimd_drain=NO_GPSIMD_DRAIN) as block,
    ):

        def emit_copy(eng, eng_spans):
            for plane, r0, r1 in eng_spans:
                if plane == "v" and PRESCATTER_VALUE:
                    # The value plane of cache_in is mutated by the value
                    # pre-scatter; copy it only afterwards.
                    eng.wait_ge(scatv_sem, 16 * (UPD_GROUPS - KEY_GROUPS))
                sem = copyk_sem if plane == "k" else copyv_sem
                eng.dma_start(
                    out=cache_out[r0:r1, :],
                    in_=cache_in[r0:r1, :],
                ).then_inc(sem, 16)

        hwdge = {"sync": block.sync, "scalar": block.scalar}
        for name, eng_spans in spans.items():
            if name == "gpsimd":
                continue  # emitted inside the gpsimd body below

            def make(sp):
                def body(eng):
                    emit_copy(eng, sp)

                return body

            hwdge[name](make(eng_spans))

        @block.gpsimd
        def _(g):
            # Stage update rows + row indices in SBUF (overlaps the copy).
            g.dma_start(out=upd_sb[:, :], in_=upd[:, :]).then_inc(load_sem, 16)
            g.dma_start(out=idx_sb[:, :], in_=idx[:, :]).then_inc(load_sem, 16)
            if "gpsimd" in spans:
                emit_copy(g, spans["gpsimd"])
            g.wait_ge(load_sem, 32)

            def scatter(j, target, sem):
                return g.indirect_dma_start(
                    out=target[:, :],
                    out_offset=bass.IndirectOffsetOnAxis(
                        ap=idx_sb[:, j : j + 1], axis=0
                    ),
                    in_=upd_sb[:, j * ROW : (j + 1) * ROW],
                    in_offset=None,
                ).then_inc(sem, 16)

            if PRESCATTER_VALUE:
                # Value rows go into cache_in up front (during the key-plane
                # copy); the value-plane copy then carries them to cache_out.
                for j in range(KEY_GROUPS, UPD_GROUPS):
                    scatter(j, cache_in, scatv_sem)
            if SPLIT_SCATTER:
                # Key rows only touch the key plane: scatter them into
                # cache_out as soon as every key-plane segment has landed,
                # overlapping the value-plane copy.
                g.wait_ge(copyk_sem, 16 * n_str)
                for j in range(KEY_GROUPS):
                    scatter(j, cache_out, scat_sem)
                if not PRESCATTER_VALUE:
                    g.wait_ge(copyv_sem, 16 * n_str)
                    for j in range(KEY_GROUPS, UPD_GROUPS):
                        scatter(j, cache_out, scat_sem)
            else:
                g.wait_ge(copyk_sem, 16 * n_str)
                g.wait_ge(copyv_sem, 16 * n_str)
                for j in range(UPD_GROUPS):
                    scatter(j, cache_out, scat_sem)
            n_out_scat = (
                KEY_GROUPS if (SPLIT_SCATTER and PRESCATTER_VALUE) else UPD_GROUPS
            )
            g.wait_ge(scat_sem, 16 * n_out_scat)
            # All value-plane copies must have landed before kernel end.
            g.wait_ge(copyv_sem, 16 * n_str)

    return nc


def _route_updates(kv_rows, local_row, core_of, shard_fallback):
    """Build per-core padded (idx, upd) tables.

    kv_rows:  (R, 2048) f32 gathered hidden rows (key half | value half)
    local_row: (R,) key-plane row index within the owning shard
    core_of:  (R,) owning core per request
    shard_fallback: per-core (key_row0_value, value_row0_value) for the
        zero-update pad case: (shard[0], shard[PLANE_ROWS]).
    Returns list of (idx[128, G] int32, upd[128, G*ROW] f32) per core.

    Layout: groups [0, KEY_GROUPS) hold key-plane entries, groups
    [KEY_GROUPS, UPD_GROUPS) hold value-plane entries, each padded with
    idempotent duplicates within its own plane.
    """
    half = MAX_UPD // 2
    out = []
    for c in range(N_CORES):
        sel = np.nonzero(core_of == c)[0]
        krows = local_row[sel]
        kvals = kv_rows[sel, :ROW]
        vrows = PLANE_ROWS + krows
        vvals = kv_rows[sel, ROW:]
        if krows.size:
            # Keep the LAST occurrence per duplicate row (sequential-write
            # semantics); reference slots are unique so this is a no-op.
            rev = krows[::-1]
            _, first_in_rev = np.unique(rev, return_index=True)
            keep = krows.size - 1 - first_in_rev
            krows, kvals = krows[keep], kvals[keep]
            vrows, vvals = vrows[keep], vvals[keep]
        n = krows.size

        idx_arr = np.empty((MAX_UPD,), np.int32)
        val_arr = np.empty((MAX_UPD, ROW), np.float32)
        if n:
            idx_arr[:n] = krows
            val_arr[:n] = kvals
            idx_arr[n:half] = krows[-1]
            val_arr[n:half] = kvals[-1]
            idx_arr[half : half + n] = vrows
            val_arr[half : half + n] = vvals
            idx_arr[half + n :] = vrows[-1]
            val_arr[half + n :] = vvals[-1]
        else:
            # No updates on this core: rewrite plane row 0 with its own value.
            k0, v0 = shard_fallback[c]
            idx_arr[:half] = 0
            val_arr[:half] = k0
            idx_arr[half:] = PLANE_ROWS
            val_arr[half:] = v0
        # Update u = j*128 + p lives at idx[p, j] / upd[p, j*ROW:(j+1)*ROW].
        idx_t = np.ascontiguousarray(idx_arr.reshape(UPD_GROUPS, 128).T)
        val_t = np.ascontiguousarray(
            val_arr.reshape(UPD_GROUPS, 128, ROW).transpose(1, 0, 2).reshape(
                128, UPD_GROUPS * ROW
            )
        )
        out.append((idx_t, val_t))
    return out


def kernel(**inputs) -> np.ndarray:
    global _NC, _NC_KEY, LAST_RESULTS

    hidden_states = np.asarray(inputs["hidden_states"], dtype=np.float32)
    kv_cache = np.asarray(inputs["kv_cache"], dtype=np.float32)
    qsl = np.asarray(inputs["query_start_loc"]).astype(np.int64)
    slot_mapping = np.asarray(inputs["slot_mapping"]).astype(np.int64)
    num_reqs = int(np.asarray(inputs["num_reqs"]))

    # Host-side routing: gather last-token rows, map slots -> (core, row).
    last = np.clip(qsl[1 : num_reqs + 1] - 1, 0, TOTAL_TOKENS - 1)
    slots = slot_mapping[last]
    blk = slots // BLOCK_SIZE
    off = slots % BLOCK_SIZE
    kv_rows = hidden_states[last]  # (R, 2048)
    core_of = blk // BLOCKS_PER_CORE
    local_row = (blk % BLOCKS_PER_CORE) * BLOCK_SIZE + off  # key-plane row

    # Shard the cache by block range; each shard viewed as (16384, 1024).
    kv3 = kv_cache.reshape(2, NUM_BLOCKS, BLOCK_SIZE * ROW)
    shards = [
        np.ascontiguousarray(
            kv3[:, c * BLOCKS_PER_CORE : (c + 1) * BLOCKS_PER_CORE]
        ).reshape(ROWS_PER_CORE, ROW)
        for c in range(N_CORES)
    ]
    shard_fallback = [
        (shards[c][0], shards[c][PLANE_ROWS]) for c in range(N_CORES)
    ]
    tables = _route_updates(kv_rows, local_row, core_of, shard_fallback)

    in_maps = [
        {"cache_in": shards[c], "upd": tables[c][1], "idx": tables[c][0]}
        for c in range(N_CORES)
    ]

    key = (COPY_STREAMS, SPLIT_SCATTER, NO_GPSIMD_DRAIN, PRESCATTER_VALUE)
    if _NC is None or _NC_KEY != key:
        _NC = _build_program()
        _NC_KEY = key

    try:
        res = bass_utils.run_bass_kernel_spmd(
            _NC, in_maps, core_ids=list(range(N_CORES)), **RUN_KWARGS
        )
    except Exception:
        # Transient NRT/device errors (e.g. NRT_EXEC_UNIT_UNRECOVERABLE right
        # after a heavy profiling run) have been observed to clear on retry.
        res = bass_utils.run_bass_kernel_spmd(
            _NC, in_maps, core_ids=list(range(N_CORES)), **RUN_KWARGS
        )
    LAST_RESULTS = res

    out = np.empty_like(kv_cache)
    out3 = out.reshape(2, NUM_BLOCKS, BLOCK_SIZE * ROW)
    for c in range(N_CORES):
        out3[:, c * BLOCKS_PER_CORE : (c + 1) * BLOCKS_PER_CORE] = res.results[c][
            "cache_out"
        ].reshape(2, BLOCKS_PER_CORE, BLOCK_SIZE * ROW)
    return out
